# revision 8
# baseline (speedup 1.0000x reference)
"""Trainium2 Bass kernel for nn_Ensembler (nms_detection).

Contract: kernel(**inputs) takes the FULL unsharded inputs
(voxel_logits [3,64,128,128,32] f32, query_logits [3,1,64,21] f32,
sem_prob_dense [21,128,128,32] f32) and returns the FULL output
[64,128,128,32] f32.

Strategy: shard the voxel grids over the flattened voxel dimension
N = X*Y*Z across 8 NeuronCores (each core owns a contiguous slice of
N).  The QxQ IoU statistics are computed as per-shard mask GEMMs
(fp8 0/1 masks on the tensor engine) reduced with a tiny AllReduce;
the argmax / matching / merge / keep steps are then replicated on
every core, and the merge + keep + occupancy masking are
embarrassingly parallel over the local N slice.  The data-dependent
row gather aux_v[aux_idx] is realized as an indirect DMA that reads
the aux logits from DRAM with device-computed row indices.

Numerical notes:
 - iteration-1 masks use sign(logit) (exact, no sigmoid involved)
 - iteration-2 anchor mask uses the algebraic identity
   (sig(x0)+sig(x1))/2 > 0.5  <=>  x0 + x1 > 0, avoiding sigmoid-LUT
   error in the decision path entirely.
 - sigmoid LUT (ScalarE) max abs error measured 3.6e-6, only affects
   output values, not decisions.
"""

import numpy as np

S = 3
Q = 64
X, Y, Z = 128, 128, 32
N = X * Y * Z           # 524288
C_SEM = 21
NCORES = 8
NS = N // NCORES        # 65536 voxels per core
JP = NS // 128          # 512 = contiguous voxels per partition (n-layout)
T = 1024                # q-layout chunk free size
NCH = NS // (2 * T)     # 32 q-layout chunks (each covers 2T voxels)
QC = 8                  # q rows per n-layout read chunk

_compiled = None


def _build_program():
    import dataclasses
    import concourse.bass as bass
    import concourse.bacc as bacc
    import concourse.mybir as mybir
    import concourse.tile as tile

    dt = mybir.dt
    Alu = mybir.AluOpType
    Act = mybir.ActivationFunctionType

    def dram_view(ap, pattern, offset_elems):
        """Raw [step,count] (element units) view of a DRAM tensor AP."""
        return dataclasses.replace(ap, ap=[list(p) for p in pattern],
                                   offset=offset_elems)

    nc = bacc.Bacc("TRN2", target_bir_lowering=False, debug=False,
                   num_devices=NCORES)

    l0 = nc.dram_tensor("l0", [Q, NS], dt.float32, kind="ExternalInput").ap()
    l1 = nc.dram_tensor("l1", [Q, NS], dt.float32, kind="ExternalInput").ap()
    l2 = nc.dram_tensor("l2", [Q, NS], dt.float32, kind="ExternalInput").ap()
    sem = nc.dram_tensor("sem", [C_SEM, NS], dt.float32,
                         kind="ExternalInput").ap()
    revcnt = nc.dram_tensor("revcnt", [Q, Q], dt.float32,
                            kind="ExternalInput").ap()
    out = nc.dram_tensor("out", [Q, NS], dt.float32,
                         kind="ExternalOutput").ap()

    with tile.TileContext(nc) as tc:
        with tc.tile_pool(name="dram", bufs=1, space="DRAM") as dramp, \
             tc.tile_pool(name="psum", bufs=1, space="PSUM") as psump, \
             tc.tile_pool(name="stats", bufs=1) as stp:

            # ---- DRAM scratch ----------------------------------------
            ma2_dram = dramp.tile([Q + 1, NS], dt.float8e4)
            m2_dram = dramp.tile([Q + 1, NS], dt.float8e4)
            occ_dram = dramp.tile([1, NS], dt.float8e4)
            cc_in1 = dramp.tile([Q + 1, Q + 1], dt.float32)
            cc_out1 = dramp.tile([Q + 1, Q + 1], dt.float32)
            cc_in2 = dramp.tile([Q + 1, Q + 1], dt.float32)
            cc_out2 = dramp.tile([Q + 1, Q + 1], dt.float32)
            pack1_dram = dramp.tile([Q, 2], dt.float32)
            pack2_dram = dramp.tile([Q, 2], dt.float32)

            # ---- small persistent stat tiles -------------------------
            revc = stp.tile([Q, Q], dt.float32)
            nc.sync.dma_start(revc[:], revcnt[:])
            iou_a1 = stp.tile([Q, 1], dt.float32)
            iou_a2 = stp.tile([Q, 1], dt.float32)
            idx1_i = stp.tile([Q, 1], dt.int32)
            idx2_i = stp.tile([Q, 1], dt.int32)
            cb_pp = stp.tile([128, 2], dt.float32)    # [0.5*matched1, matched1]
            c3k_pp = stp.tile([128, 2], dt.float32)   # [matched2/3, keep]

            # =========================================================
            # PASS A: masks m0,m1 -> G1 GEMM; m2 -> DRAM; occupancy
            # =========================================================
            g1_ps = psump.tile([Q + 1, Q + 1], dt.float32)
            with tc.tile_pool(name="passa", bufs=1) as pa:
                m0_sb = pa.tile([128, Q + 1, JP], dt.float8e4)
                m1_sb = pa.tile([128, Q + 1, JP], dt.float8e4)
                nc.vector.memset(m0_sb[:, Q, :], 1.0)
                nc.vector.memset(m1_sb[:, Q, :], 1.0)
                # n-layout q-chunked reads: n = p*JP + j
                for qc in range(Q // QC):
                    for (lsrc, msb) in ((l0, m0_sb), (l1, m1_sb)):
                        lc = pa.tile([128, QC, JP], dt.float32, tag="ldchunk",
                                     bufs=3)
                        src = dram_view(lsrc, [[JP, 128], [NS, QC], [1, JP]],
                                        qc * QC * NS)
                        nc.sync.dma_start(lc[:], src)
                        nc.vector.tensor_scalar(
                            msb[:, qc * QC:(qc + 1) * QC, :], lc[:], 0.0,
                            None, op0=Alu.is_gt)
                # G1 GEMM: accumulate over all 512 j-slices
                for j in range(JP):
                    nc.tensor.matmul(g1_ps[:], lhsT=m0_sb[:, :, j],
                                     rhs=m1_sb[:, :, j],
                                     start=(j == 0), stop=(j == JP - 1))
                # m2 masks -> DRAM (n-layout write), ones row included
                ones_c = pa.tile([128, JP], dt.float8e4)
                nc.vector.memset(ones_c[:], 1.0)
                nc.sync.dma_start(
                    dram_view(m2_dram, [[JP, 128], [1, JP]], Q * NS),
                    ones_c[:])
                for qc in range(Q // QC):
                    lc2 = pa.tile([128, QC, JP], dt.float32, tag="ldchunk",
                                  bufs=3)
                    src = dram_view(l2, [[JP, 128], [NS, QC], [1, JP]],
                                    qc * QC * NS)
                    nc.sync.dma_start(lc2[:], src)
                    m2c = pa.tile([128, QC, JP], dt.float8e4, tag="m2chunk",
                                  bufs=2)
                    nc.vector.tensor_scalar(m2c[:], lc2[:], 0.0, None,
                                            op0=Alu.is_gt)
                    dst = dram_view(m2_dram, [[JP, 128], [NS, QC], [1, JP]],
                                    qc * QC * NS)
                    nc.sync.dma_start(dst, m2c[:])
                # occupancy: occ[n] = (max_{c>=1} sem[c,n] > sem[0,n])
                sem0 = pa.tile([128, JP], dt.float32)
                nc.sync.dma_start(sem0[:],
                                  dram_view(sem, [[JP, 128], [1, JP]], 0))
                mx = pa.tile([128, JP], dt.float32)
                nc.sync.dma_start(mx[:],
                                  dram_view(sem, [[JP, 128], [1, JP]], NS))
                for c in range(2, C_SEM):
                    semc = pa.tile([128, JP], dt.float32, tag="semc", bufs=3)
                    nc.sync.dma_start(
                        semc[:], dram_view(sem, [[JP, 128], [1, JP]], c * NS))
                    nc.vector.tensor_tensor(mx[:], mx[:], semc[:], op=Alu.max)
                occ_n = pa.tile([128, JP], dt.float8e4)
                nc.vector.tensor_tensor(occ_n[:], mx[:], sem0[:], op=Alu.is_gt)
                nc.sync.dma_start(
                    dram_view(occ_dram, [[JP, 128], [1, JP]], 0), occ_n[:])

            # ---- AllReduce G1 + stats for iteration 1 ----------------
            def stats_round(g_ps, cc_in, cc_out, iou_a, idx_i):
                """Returns matched [Q,1] f32 tile (in stats pool)."""
                gs = stp.tile([Q + 1, Q + 1], dt.float32, name=f"gs_{cc_in.name}")
                nc.vector.tensor_copy(gs[:], g_ps[:])
                nc.sync.dma_start(cc_in[:], gs[:])
                nc.gpsimd.collective_compute(
                    "AllReduce", Alu.add,
                    replica_groups=[list(range(NCORES))],
                    ins=[cc_in.opt()], outs=[cc_out.opt()])
                gr = stp.tile([Q + 1, Q + 1], dt.float32, name=f"gr_{cc_in.name}")
                nc.sync.dma_start(gr[:], cc_out[:])
                # broadcast row Q (= sb) across 64 partitions
                sbb = stp.tile([Q, Q], dt.float32, name=f"sbb_{cc_in.name}")
                row = cc_out[Q:Q + 1, 0:Q]
                nc.sync.dma_start(
                    sbb[:], dataclasses.replace(
                        row, ap=[[0, Q]] + [list(p) for p in row.ap[1:]]))
                inter = gr[0:Q, 0:Q]
                sa = gr[0:Q, Q:Q + 1]
                u = stp.tile([Q, Q], dt.float32, name=f"u_{cc_in.name}")
                nc.vector.tensor_scalar(u[:], inter, sa, None, op0=Alu.subtract)
                nc.vector.tensor_tensor(u[:], sbb[:], u[:], op=Alu.subtract)
                nc.vector.tensor_scalar(u[:], u[:], 1.0, None, op0=Alu.max)
                nc.vector.reciprocal(u[:], u[:])
                iou = stp.tile([Q, Q], dt.float32, name=f"iou_{cc_in.name}")
                nc.vector.tensor_tensor(iou[:], inter, u[:], op=Alu.mult)
                nc.vector.tensor_reduce(iou_a[:], iou[:],
                                        axis=mybir.AxisListType.X, op=Alu.max)
                matched = stp.tile([Q, 1], dt.float32, name=f"mt_{cc_in.name}")
                nc.vector.tensor_scalar(matched[:], iou_a[:], 0.2, None,
                                        op0=Alu.is_gt)
                eq = stp.tile([Q, Q], dt.float32, name=f"eq_{cc_in.name}")
                nc.vector.tensor_scalar(eq[:], iou[:], iou_a[:, 0:1], None,
                                        op0=Alu.is_equal)
                nc.vector.tensor_tensor(eq[:], eq[:], revc[:], op=Alu.mult)
                sm = stp.tile([Q, 1], dt.float32, name=f"sm_{cc_in.name}")
                nc.vector.tensor_reduce(sm[:], eq[:],
                                        axis=mybir.AxisListType.X, op=Alu.max)
                # idx = Q - sm
                nc.vector.tensor_scalar(sm[:], sm[:], -1.0, float(Q),
                                        op0=Alu.mult, op1=Alu.add)
                nc.vector.tensor_copy(idx_i[:], sm[:])
                return matched

            matched1 = stats_round(g1_ps, cc_in1, cc_out1, iou_a1, idx1_i)
            # [0.5*matched1, matched1] replicated to 128 partitions via DRAM
            cb64 = stp.tile([Q, 2], dt.float32)
            nc.vector.tensor_scalar(cb64[:, 0:1], matched1[:], 0.5, None,
                                    op0=Alu.mult)
            nc.vector.tensor_copy(cb64[:, 1:2], matched1[:])
            nc.sync.dma_start(pack1_dram[:], cb64[:])
            nc.sync.dma_start(
                cb_pp[:], dram_view(pack1_dram, [[0, 2], [2, Q], [1, 2]], 0))

            # =========================================================
            # PASS B: anchor2 blend (q-layout) + ma2 mask -> G2 GEMM
            # =========================================================
            g2_ps = psump.tile([Q + 1, Q + 1], dt.float32)
            with tc.tile_pool(name="persist", bufs=1) as pp:
                anchor2 = pp.tile([128, NS // 2], dt.float32)

                with tc.tile_pool(name="blend", bufs=1) as pb:
                    for ci in range(NCH):
                        l0c = pb.tile([128, T], dt.float32, tag="l0c", bufs=3)
                        nc.sync.dma_start(
                            l0c[:], dram_view(l0, [[T, 2], [NS, Q], [1, T]],
                                              ci * 2 * T))
                        l1gc = pb.tile([128, T], dt.float32, tag="l1gc",
                                       bufs=3)
                        for qb in range(2):
                            nc.gpsimd.indirect_dma_start(
                                out=l1gc[qb * Q:(qb + 1) * Q, :],
                                out_offset=None, in_=l1[:],
                                in_offset=bass.IndirectOffsetOnAxis(
                                    ap=idx1_i[:, :1], axis=0),
                                element_offset=ci * 2 * T + qb * T)
                        p0 = pb.tile([128, T], dt.float32, tag="p0", bufs=2)
                        nc.scalar.activation(p0[:], l0c[:], Act.Sigmoid)
                        p1g = pb.tile([128, T], dt.float32, tag="p1g", bufs=2)
                        nc.scalar.activation(p1g[:], l1gc[:], Act.Sigmoid)
                        d = pb.tile([128, T], dt.float32, tag="d", bufs=2)
                        nc.vector.tensor_tensor(d[:], p1g[:], p0[:],
                                                op=Alu.subtract)
                        a2s = anchor2[:, ci * T:(ci + 1) * T]
                        nc.vector.scalar_tensor_tensor(
                            a2s, d[:], cb_pp[:, 0:1], p0[:],
                            op0=Alu.mult, op1=Alu.add)
                        # exact mask: (l0 + matched1*l1g) > 0
                        zz = pb.tile([128, T], dt.float32, tag="zz", bufs=2)
                        nc.vector.scalar_tensor_tensor(
                            zz[:], l1gc[:], cb_pp[:, 1:2], l0c[:],
                            op0=Alu.mult, op1=Alu.add)
                        ma2c = pb.tile([128, T], dt.float8e4, tag="ma2c",
                                       bufs=2)
                        nc.vector.tensor_scalar(ma2c[:], zz[:], 0.0, None,
                                                op0=Alu.is_gt)
                        for qb in range(2):
                            dst = dram_view(
                                ma2_dram, [[NS, Q], [1, T]],
                                ci * 2 * T + qb * T)
                            nc.sync.dma_start(dst,
                                              ma2c[qb * Q:(qb + 1) * Q, :])
                    # ones row for ma2
                    ones_r = pb.tile([128, JP], dt.float8e4)
                    nc.vector.memset(ones_r[:], 1.0)
                    nc.sync.dma_start(
                        dram_view(ma2_dram, [[JP, 128], [1, JP]], Q * NS),
                        ones_r[:])

                # ---- G2 GEMM from DRAM round-trip masks --------------
                with tc.tile_pool(name="g2", bufs=1) as pg:
                    ma2t = pg.tile([128, Q + 1, JP], dt.float8e4)
                    m2t = pg.tile([128, Q + 1, JP], dt.float8e4)
                    for h in range(4):
                        sl = slice(h * (JP // 4), (h + 1) * (JP // 4))
                        nc.sync.dma_start(
                            ma2t[:, :, sl],
                            dram_view(ma2_dram,
                                      [[JP, 128], [NS, Q + 1], [1, JP // 4]],
                                      h * (JP // 4)))
                        nc.sync.dma_start(
                            m2t[:, :, sl],
                            dram_view(m2_dram,
                                      [[JP, 128], [NS, Q + 1], [1, JP // 4]],
                                      h * (JP // 4)))
                    for j in range(JP):
                        nc.tensor.matmul(g2_ps[:], lhsT=ma2t[:, :, j],
                                         rhs=m2t[:, :, j],
                                         start=(j == 0), stop=(j == JP - 1))

                matched2 = stats_round(g2_ps, cc_in2, cc_out2, iou_a2, idx2_i)
                # keep = (0.5*(iou_a1+iou_a2) > 0.2); c3 = matched2/3
                pk = stp.tile([Q, 2], dt.float32)
                nc.vector.tensor_scalar(pk[:, 0:1], matched2[:], 1.0 / 3.0,
                                        None, op0=Alu.mult)
                t64 = stp.tile([Q, 1], dt.float32)
                nc.vector.tensor_tensor(t64[:], iou_a1[:], iou_a2[:],
                                        op=Alu.add)
                nc.vector.tensor_scalar(pk[:, 1:2], t64[:], 0.5, 0.2,
                                        op0=Alu.mult, op1=Alu.is_gt)
                nc.sync.dma_start(pack2_dram[:], pk[:])
                nc.sync.dma_start(
                    c3k_pp[:],
                    dram_view(pack2_dram, [[0, 2], [2, Q], [1, 2]], 0))

                # =====================================================
                # PASS C: final merge + keep + occupancy -> out
                # =====================================================
                with tc.tile_pool(name="passc", bufs=1) as pc:
                    for ci in range(NCH):
                        l2gc = pc.tile([128, T], dt.float32, tag="l2gc",
                                       bufs=3)
                        for qb in range(2):
                            nc.gpsimd.indirect_dma_start(
                                out=l2gc[qb * Q:(qb + 1) * Q, :],
                                out_offset=None, in_=l2[:],
                                in_offset=bass.IndirectOffsetOnAxis(
                                    ap=idx2_i[:, :1], axis=0),
                                element_offset=ci * 2 * T + qb * T)
                        p2g = pc.tile([128, T], dt.float32, tag="p2g", bufs=2)
                        nc.scalar.activation(p2g[:], l2gc[:], Act.Sigmoid)
                        a2s = anchor2[:, ci * T:(ci + 1) * T]
                        d2 = pc.tile([128, T], dt.float32, tag="d2", bufs=2)
                        nc.vector.tensor_tensor(d2[:], p2g[:], a2s,
                                                op=Alu.subtract)
                        sm2 = pc.tile([128, T], dt.float32, tag="sm2", bufs=2)
                        nc.vector.scalar_tensor_tensor(
                            sm2[:], d2[:], c3k_pp[:, 0:1], a2s,
                            op0=Alu.mult, op1=Alu.add)
                        nc.vector.tensor_scalar(sm2[:], sm2[:],
                                                c3k_pp[:, 1:2], None,
                                                op0=Alu.mult)
                        occb = pc.tile([128, T], dt.float8e4, tag="occb",
                                       bufs=2)
                        for qb in range(2):
                            nc.sync.dma_start(
                                occb[qb * Q:(qb + 1) * Q, :],
                                dram_view(occ_dram, [[0, Q], [1, T]],
                                          ci * 2 * T + qb * T))
                        oc = pc.tile([128, T], dt.float32, tag="oc", bufs=2)
                        nc.vector.tensor_tensor(oc[:], sm2[:], occb[:],
                                                op=Alu.mult)
                        nc.sync.dma_start(
                            dram_view(out, [[T, 2], [NS, Q], [1, T]],
                                      ci * 2 * T),
                            oc[:])

    nc.compile()
    return nc


def _get_program():
    global _compiled
    if _compiled is None:
        _compiled = _build_program()
    return _compiled


def _make_in_maps(voxel_logits, sem_prob_dense):
    vl = np.ascontiguousarray(
        np.asarray(voxel_logits, dtype=np.float32).reshape(S, Q, N))
    sp = np.ascontiguousarray(
        np.asarray(sem_prob_dense, dtype=np.float32).reshape(C_SEM, N))
    revcnt = np.tile((Q - np.arange(Q, dtype=np.float32))[None, :], (Q, 1))
    in_maps = []
    for c in range(NCORES):
        sl = slice(c * NS, (c + 1) * NS)
        in_maps.append({
            "l0": np.ascontiguousarray(vl[0, :, sl]),
            "l1": np.ascontiguousarray(vl[1, :, sl]),
            "l2": np.ascontiguousarray(vl[2, :, sl]),
            "sem": np.ascontiguousarray(sp[:, sl]),
            "revcnt": revcnt,
        })
    return in_maps


def profile_run(inputs):
    """Run once with NTFF tracing; returns exec_time_ns or None."""
    from concourse.bass_utils import run_bass_kernel_spmd

    nc = _get_program()
    in_maps = _make_in_maps(inputs["voxel_logits"], inputs["sem_prob_dense"])
    res = run_bass_kernel_spmd(nc, in_maps, list(range(NCORES)), trace=True)
    return res.exec_time_ns


def kernel(voxel_logits, query_logits, sem_prob_dense):
    from concourse.bass_utils import run_bass_kernel_spmd

    nc = _get_program()
    in_maps = _make_in_maps(voxel_logits, sem_prob_dense)
    res = run_bass_kernel_spmd(nc, in_maps, list(range(NCORES)))
    full = np.concatenate([res.results[c]["out"] for c in range(NCORES)],
                          axis=1)
    return full.reshape(Q, X, Y, Z).astype(np.float32)
